# revision 13
# baseline (speedup 1.0000x reference)
"""Trainium2 Bass kernel for the BoxDetectionLoss problem.

Contract: kernel(**inputs) takes the FULL inputs
    policy_output (32, 12, 256, 256) f32
    target_probs  (32, 64)           f32
    target_boxes  (32, 64, 4)        i32
    target_mask   (32, 64)           bool
and returns the FULL scalar loss (f32), matching reference().

Strategy: pure data parallel over batch across 8 NeuronCores (4 batches
per core). Each core computes an unnormalized partial sum of the three
loss terms for its batch shard; the host adds the 8 partials and divides
by B*H*W*3.

Math used on device (per core, Bs=4 batches, T=64 targets, 3 anchors):
  validity_loss = sum softplus(v) over ALL positions
                  + sum over matched positions of (-v)
     [since softplus(-v) - softplus(v) = -v]
  matched(b,t,a) at position (r,c)=(tr1,tc1):
     round(clip(tr1+9*sig(dr),0,255)) == tr2
     and round(clip(tc1+16*sig(dc),0,255)) == tc2 and mask
     and no earlier target t'<t with the identical box and mask[t']
     (the dedup reproduces the reference's first-match-wins scan,
      which counts each matched position once)
  coord = |pred_r2 - tr2| + |pred_c2 - tc2|, pref = (sig(p) - prob)^2

Only the 3 validity channels are streamed densely (25% of the input
bytes — the rest of policy_output is never read in full). The sparse
per-target values po[b, ch, tr1, tc1] come from an indirect (vector-
dynamic-offset) DMA gather of 64-byte chunks at device-computed
offsets, followed by a within-chunk mask-select. softplus/sigmoid are
built from exp/ln/reciprocal so the kernel needs a single ACT table set
(natural_log_exp_and_others).
"""

import sys

for _p in ("/opt/trn_rl_repo", "/root/.axon_site/_ro/trn_rl_repo"):
    if _p not in sys.path:
        sys.path.insert(0, _p)

import numpy as np

import bass_rust
import concourse.bass as bass
import concourse.bass_utils as bass_utils_mod
import concourse.mybir as mybir
import concourse.tile as tile_mod
from concourse.bass_utils import run_bass_kernel_spmd
from concourse.masks import make_identity
from concourse.tile import ScopedClock, TileContext

AF = mybir.ActivationFunctionType
ALU = mybir.AluOpType
F32 = mybir.dt.float32
I32 = mybir.dt.int32
U8 = mybir.dt.uint8

N_CORES = 8
B, C, H, W = 32, 12, 256, 256
T = 64
BS = B // N_CORES  # 4 batches per core
NORM = B * H * W * 3
MAGIC = 8388608.0  # 2^23: (x + 2^23) - 2^23 rounds f32 to nearest-even int
CH = 16  # gather chunk: 16 f32 = 64B


# ---------------------------------------------------------------------------
# Environment workarounds for the pinned walrus build:
# 1) it rejects instructions carrying more than ~1 sem wait, while Tile's
#    scheduler and kernel-tail drain attach several;
# 2) dynamic (indirect) DMA lowering needs --dge-levels, which bass_utils
#    doesn't pass.
# ---------------------------------------------------------------------------
def _patched_drain_and_barrier(self, tick_clock, wait_clock):
    nc = self.nc
    gc = tick_clock.global_clock
    ticks = list(gc)
    n = len(ticks)
    cur = ScopedClock({None: bass_rust.VectorClock([0] * n)})
    for i, v in enumerate(ticks):
        if v > 0:
            nop = nc.sync.nop(nofuse=True)
            single = ScopedClock(
                {None: bass_rust.VectorClock([v if j == i else 0 for j in range(n)])}
            )
            wait_clock.add_sem_waits(nop.ins, single, cur)
            cur.update_past(single)
    drain_inst = nc.sync.drain()
    wait_clock.add_sem_waits(drain_inst.ins, ScopedClock({None: gc}), cur)

    nc.all_engine_barrier()
    assert self.sems is not None
    popped = nc._tile_sem_poison_stack.pop()
    assert popped is self._sem_poison
    nc.clear_and_free_semaphores(list(self.sems.allocated().values()))
    nc.all_engine_barrier()


tile_mod.TileContext._drain_and_barrier = _patched_drain_and_barrier

_orig_lower_ordered_insts = tile_mod.TileContext._lower_ordered_insts


def _split_excess_waits(self, ordered):
    nc = self.nc
    for bb_name, insts in ordered.items():
        new_insts = []
        for inst in insts:
            si = getattr(inst, "sync_info", None)
            waits = list(si.on_wait) if si is not None else []
            if len(waits) > 1:
                for w in waits[:-1]:
                    nop = mybir.InstNoOp(name=nc.get_next_instruction_name())
                    nop.engine = inst.engine
                    nop.sync_info = mybir.SyncInfo(on_wait=[w], on_update=[])
                    new_insts.append(nop)
                inst.sync_info = mybir.SyncInfo(
                    on_wait=[waits[-1]], on_update=list(si.on_update)
                )
            new_insts.append(inst)
        if len(new_insts) != len(insts):
            insts[:] = new_insts
    return _orig_lower_ordered_insts(self, ordered)


tile_mod.TileContext._lower_ordered_insts = _split_excess_waits

_orig_run_command = bass_utils_mod.run_command


def _patched_run_command(argv, **kw):
    argv = list(argv)
    if argv and "walrus_driver" in str(argv[0]):
        argv.append(
            "--dge-levels=io,spill_reload,scalar_dynamic_offset,"
            "vector_dynamic_offsets"
        )
    return _orig_run_command(argv, **kw)


bass_utils_mod.run_command = _patched_run_command


def build_kernel(loop_k: int | None = None, gather_proxy: bool = False) -> bass.Bass:
    """loop_k: benchmark mode — repeat the computation loop_k times inside
    a Tile For_i. Indirect DMA does not lower inside For_i on this
    toolchain, so benchmark mode sets gather_proxy=True and replaces the
    gathers with static DMAs of identical descriptor structure
    (768 x 64B scattered reads). Timing only — results are wrong with
    the proxy."""
    import contextlib

    nc = bass.Bass()

    po_d = nc.dram_tensor("po", [BS, C, H, W], F32, kind="ExternalInput")
    boxes_d = nc.dram_tensor("boxes", [BS, T, 4], I32, kind="ExternalInput")
    probs_d = nc.dram_tensor("probs", [BS, T], F32, kind="ExternalInput")
    mask_d = nc.dram_tensor("mask", [BS, T], U8, kind="ExternalInput")
    out_d = nc.dram_tensor("out", [1, 1], F32, kind="ExternalOutput")

    with TileContext(nc) as tc:
        with (
            tc.tile_pool(name="const", bufs=1) as cpool,
            tc.tile_pool(name="po", bufs=6) as popool,
            tc.tile_pool(name="expv", bufs=6) as vpool,
            tc.tile_pool(name="g16", bufs=4) as gpool,
            tc.tile_pool(name="scr", bufs=4) as spool,
            tc.tile_pool(name="ptr", bufs=2, space="PSUM") as ptpool,
            tc.tile_pool(name="tot", bufs=1, space="PSUM") as totpool,
        ):
            # ---- constants -------------------------------------------------
            ident = cpool.tile([64, 64], F32)
            make_identity(nc, ident[:])

            iota_p_i = cpool.tile([64, 1], I32)
            nc.gpsimd.iota(iota_p_i[:], pattern=[[0, 1]], base=0, channel_multiplier=1)
            iota_pf = cpool.tile([64, 1], F32)
            nc.vector.tensor_copy(iota_pf[:], iota_p_i[:])

            iota_t_i = cpool.tile([64, T], I32)
            nc.gpsimd.iota(iota_t_i[:], pattern=[[1, T]], base=0, channel_multiplier=0)
            iota_tf = cpool.tile([64, T], F32)
            nc.vector.tensor_copy(iota_tf[:], iota_t_i[:])
            # strict lower triangle: lt[p, f] = (f < p)
            ltm = cpool.tile([64, T], F32)
            nc.vector.tensor_scalar(
                ltm[:], iota_tf[:], iota_pf[:], None, op0=ALU.is_lt
            )

            ones_col = cpool.tile([128, 1], F32)
            nc.gpsimd.memset(ones_col[:], 1.0)

            # gather-offset building blocks
            base12_i = cpool.tile([64, C], I32)  # ch * H * CH
            nc.gpsimd.iota(
                base12_i[:], pattern=[[H * CH, C]], base=0, channel_multiplier=0
            )
            base12 = cpool.tile([64, C], F32)
            nc.vector.tensor_copy(base12[:], base12_i[:])
            iota16_i = cpool.tile([64, C, CH], I32)  # 12 repeats of 0..15
            nc.gpsimd.iota(
                iota16_i[:], pattern=[[0, C], [1, CH]], base=0, channel_multiplier=0
            )
            iota16f = cpool.tile([64, C, CH], F32)
            nc.vector.tensor_copy(iota16f[:], iota16_i[:])

            # ---- targets ---------------------------------------------------
            boxes_i = cpool.tile([T, BS, 4], I32)
            nc.sync.dma_start(out=boxes_i[:], in_=boxes_d.rearrange("b t i -> t b i"))
            boxes_f = cpool.tile([T, BS, 4], F32)
            nc.vector.tensor_copy(boxes_f[:], boxes_i[:])
            tr1 = boxes_f[:, :, 0]
            tc1 = boxes_f[:, :, 1]
            tr2 = boxes_f[:, :, 2]
            tc2 = boxes_f[:, :, 3]

            probs_f = cpool.tile([T, BS], F32)
            nc.sync.dma_start(out=probs_f[:], in_=probs_d.rearrange("b t -> t b"))
            mask_u = cpool.tile([T, BS], U8)
            nc.sync.dma_start(out=mask_u[:], in_=mask_d.rearrange("b t -> t b"))
            mask_f = cpool.tile([T, BS], F32)
            nc.vector.tensor_copy(mask_f[:], mask_u[:])

            # per-target chunk offset pieces: tadd = tr1*16 + tc1>>4,
            # within-chunk position tcmod = tc1 & 15
            t16 = cpool.tile([T, BS], I32)
            nc.vector.tensor_scalar(
                t16[:], boxes_i[:, :, 0], 4, None, op0=ALU.arith_shift_left
            )
            tc4 = cpool.tile([T, BS], I32)
            nc.vector.tensor_scalar(
                tc4[:], boxes_i[:, :, 1], 4, None, op0=ALU.arith_shift_right
            )
            tadd = cpool.tile([T, BS], I32)
            nc.vector.tensor_tensor(tadd[:], t16[:], tc4[:], op=ALU.add)
            tadd_f = cpool.tile([T, BS], F32)
            nc.vector.tensor_copy(tadd_f[:], tadd[:])
            tcm_i = cpool.tile([T, BS], I32)
            nc.vector.tensor_scalar(
                tcm_i[:], boxes_i[:, :, 1], 15, None, op0=ALU.bitwise_and
            )
            tcmod_f = cpool.tile([T, BS], F32)
            nc.vector.tensor_copy(tcmod_f[:], tcm_i[:])

            # box keys for duplicate suppression (each fits f32 exactly)
            k1 = cpool.tile([T, BS], F32)
            nc.vector.scalar_tensor_tensor(
                k1[:], tr1, 256.0, tc1, op0=ALU.mult, op1=ALU.add
            )
            k2 = cpool.tile([T, BS], F32)
            nc.vector.scalar_tensor_tensor(
                k2[:], tr2, 256.0, tc2, op0=ALU.mult, op1=ALU.add
            )

            # ---- phase 0: dedup gate ---------------------------------------
            supp = cpool.tile([T, BS], F32)
            for b in range(BS):
                k1t_ps = ptpool.tile([64, 64], F32, tag="tp")
                nc.tensor.transpose(
                    out=k1t_ps[:], in_=k1[:, b : b + 1].to_broadcast([64, 64]),
                    identity=ident[:],
                )
                k2t_ps = ptpool.tile([64, 64], F32, tag="tp")
                nc.tensor.transpose(
                    out=k2t_ps[:], in_=k2[:, b : b + 1].to_broadcast([64, 64]),
                    identity=ident[:],
                )
                mt_ps = ptpool.tile([64, 64], F32, tag="tp")
                nc.tensor.transpose(
                    out=mt_ps[:], in_=mask_f[:, b : b + 1].to_broadcast([64, 64]),
                    identity=ident[:],
                )
                eq1 = spool.tile([64, 64], F32, tag="eq1")
                nc.vector.tensor_scalar(
                    eq1[:], k1t_ps[:], k1[:, b : b + 1], None, op0=ALU.is_equal
                )
                eq2 = spool.tile([64, 64], F32, tag="eq2")
                nc.vector.tensor_scalar(
                    eq2[:], k2t_ps[:], k2[:, b : b + 1], None, op0=ALU.is_equal
                )
                both = spool.tile([64, 64], F32, tag="both")
                nc.vector.tensor_tensor(both[:], eq1[:], eq2[:], op=ALU.mult)
                gatet = spool.tile([64, 64], F32, tag="gatet")
                nc.vector.tensor_tensor(gatet[:], ltm[:], mt_ps[:], op=ALU.mult)
                scr64 = spool.tile([64, 64], F32, tag="scr64")
                nc.vector.tensor_tensor(scr64[:], both[:], gatet[:], op=ALU.mult)
                nc.vector.tensor_reduce(
                    supp[:, b : b + 1], scr64[:], axis=mybir.AxisListType.X, op=ALU.max
                )

            # gate[t, b] = mask[t, b] * (1 - supp[t, b])
            gate = cpool.tile([T, BS], F32)
            nc.vector.tensor_scalar(
                gate[:], supp[:], -1.0, 1.0, op0=ALU.mult, op1=ALU.add
            )
            nc.vector.tensor_tensor(gate[:], gate[:], mask_f[:], op=ALU.mult)

            # ---- phases 1+2, emitted once (or loop_k times for timing) -----
            acc_dense = cpool.tile([128, 6], F32)
            G = cpool.tile([64, C, BS], F32)  # gathered po[b, ch, tr1, tc1]
            src_chunks = po_d.rearrange("b c h (s x) -> (b c h s) x", x=CH)

            def emit_body():
                # sparse gathers first: tiny, independent of the big stream
                for b in range(BS):
                    off_f = spool.tile([64, C], F32, tag="offf")
                    nc.vector.tensor_scalar(
                        off_f[:], base12[:], tadd_f[:, b : b + 1],
                        float(b * C * H * CH), op0=ALU.add, op1=ALU.add,
                    )
                    off_b = spool.tile([64, C], I32, tag="off")
                    nc.vector.tensor_copy(off_b[:], off_f[:])
                    g16 = gpool.tile([64, C, CH], F32, tag="g16")
                    if gather_proxy:
                        nc.sync.dma_start(
                            out=g16[:],
                            in_=po_d.rearrange(
                                "b c h (s x) -> h b c s x", x=CH
                            )[0:64, 0, :, 0, :],
                        )
                    else:
                        nc.gpsimd.indirect_dma_start(
                            out=g16[:], out_offset=None, in_=src_chunks,
                            in_offset=bass.IndirectOffsetOnAxis(ap=off_b[:], axis=0),
                        )
                    m_b = spool.tile([64, C, CH], F32, tag="m16")
                    nc.vector.tensor_scalar(
                        m_b[:], iota16f[:], tcmod_f[:, b : b + 1], None,
                        op0=ALU.is_equal,
                    )
                    prod = spool.tile([64, C, CH], F32, tag="gprod")
                    nc.vector.tensor_tensor(prod[:], g16[:], m_b[:], op=ALU.mult)
                    nc.vector.tensor_reduce(
                        G[:, :, b], prod[:], axis=mybir.AxisListType.X, op=ALU.add
                    )

                # dense stream: one DMA per (anchor, row-half) = 512KB each
                for a in range(3):
                    for h in range(2):
                        pab = popool.tile([128, BS, W], F32, tag="pob")
                        dma_eng = nc.sync if (2 * a + h) % 2 == 0 else nc.scalar
                        dma_eng.dma_start(
                            out=pab[:],
                            in_=po_d[:, 4 * a + 2].rearrange(
                                "b (n p) w -> p b n w", p=128
                            )[:, :, h],
                        )
                        ve = vpool.tile([128, BS, W], F32, tag="ve")
                        nc.scalar.activation(out=ve[:], in_=pab[:], func=AF.Exp)
                        nc.scalar.activation(
                            out=ve[:], in_=ve[:], func=AF.Ln, bias=1.0,
                            accum_out=acc_dense[:, 2 * a + h : 2 * a + h + 1],
                        )

                # ---- phase 2: per-target loss terms ------------------------
                sg = cpool.tile([64, C, BS], F32)
                gflat = G[:].rearrange("p c b -> p (c b)")
                sgflat = sg[:].rearrange("p c b -> p (c b)")
                nc.scalar.activation(out=sgflat, in_=gflat, func=AF.Exp, scale=-1.0)
                nc.vector.tensor_scalar_add(sgflat, sgflat, 1.0)
                nc.vector.reciprocal(sgflat, sgflat)

                acc_sparse = cpool.tile([64, BS], F32)
                nc.gpsimd.memset(acc_sparse[:], 0.0)

                for a in range(3):
                    drs = sg[:, 4 * a + 0, :]
                    dcs = sg[:, 4 * a + 1, :]
                    vv = G[:, 4 * a + 2, :]
                    ps = sg[:, 4 * a + 3, :]

                    pr2 = spool.tile([64, BS], F32, tag="pr2")
                    nc.vector.scalar_tensor_tensor(
                        pr2[:], drs, 9.0, tr1, op0=ALU.mult, op1=ALU.add
                    )
                    nc.vector.tensor_scalar_min(pr2[:], pr2[:], float(H - 1))
                    pc2 = spool.tile([64, BS], F32, tag="pc2")
                    nc.vector.scalar_tensor_tensor(
                        pc2[:], dcs, 16.0, tc1, op0=ALU.mult, op1=ALU.add
                    )
                    nc.vector.tensor_scalar_min(pc2[:], pc2[:], float(W - 1))

                    rnd_r = spool.tile([64, BS], F32, tag="rnd_r")
                    nc.vector.tensor_scalar(
                        rnd_r[:], pr2[:], MAGIC, -MAGIC, op0=ALU.add, op1=ALU.add
                    )
                    rnd_c = spool.tile([64, BS], F32, tag="rnd_c")
                    nc.vector.tensor_scalar(
                        rnd_c[:], pc2[:], MAGIC, -MAGIC, op0=ALU.add, op1=ALU.add
                    )

                    mr = spool.tile([64, BS], F32, tag="mr")
                    nc.vector.tensor_tensor(mr[:], rnd_r[:], tr2, op=ALU.is_equal)
                    mc = spool.tile([64, BS], F32, tag="mc")
                    nc.vector.tensor_tensor(mc[:], rnd_c[:], tc2, op=ALU.is_equal)
                    matched = spool.tile([64, BS], F32, tag="matched")
                    nc.vector.tensor_tensor(matched[:], mr[:], mc[:], op=ALU.mult)
                    nc.vector.tensor_tensor(
                        matched[:], matched[:], gate[:], op=ALU.mult
                    )

                    d1 = spool.tile([64, BS], F32, tag="d1")
                    nc.vector.tensor_tensor(d1[:], pr2[:], tr2, op=ALU.subtract)
                    nc.scalar.activation(out=d1[:], in_=d1[:], func=AF.Abs)
                    d2 = spool.tile([64, BS], F32, tag="d2")
                    nc.vector.tensor_tensor(d2[:], pc2[:], tc2, op=ALU.subtract)
                    nc.scalar.activation(out=d2[:], in_=d2[:], func=AF.Abs)

                    dp = spool.tile([64, BS], F32, tag="dp")
                    nc.vector.tensor_tensor(dp[:], ps, probs_f[:], op=ALU.subtract)
                    nc.vector.tensor_tensor(dp[:], dp[:], dp[:], op=ALU.mult)

                    s = spool.tile([64, BS], F32, tag="s")
                    nc.vector.tensor_tensor(s[:], d1[:], d2[:], op=ALU.add)
                    nc.vector.tensor_tensor(s[:], s[:], dp[:], op=ALU.add)
                    nc.vector.tensor_tensor(s[:], s[:], vv, op=ALU.subtract)
                    nc.vector.tensor_tensor(s[:], s[:], matched[:], op=ALU.mult)
                    nc.vector.tensor_tensor(
                        acc_sparse[:], acc_sparse[:], s[:], op=ALU.add
                    )

                # ---- final reduction ---------------------------------------
                dn_red = cpool.tile([128, 1], F32)
                nc.vector.tensor_reduce(
                    dn_red[:], acc_dense[:], axis=mybir.AxisListType.X, op=ALU.add
                )
                sp_red = cpool.tile([64, 1], F32)
                nc.vector.tensor_reduce(
                    sp_red[:], acc_sparse[:], axis=mybir.AxisListType.X, op=ALU.add
                )
                tot_ps = totpool.tile([1, 1], F32, tag="tot")
                nc.tensor.matmul(
                    out=tot_ps[:], lhsT=dn_red[:], rhs=ones_col[:],
                    start=True, stop=False,
                )
                nc.tensor.matmul(
                    out=tot_ps[:], lhsT=sp_red[:], rhs=ones_col[:64, :],
                    start=False, stop=True,
                )
                out_sb = cpool.tile([1, 1], F32)
                nc.vector.tensor_copy(out_sb[:], tot_ps[:])
                nc.sync.dma_start(out=out_d[:], in_=out_sb[:])

            if loop_k:
                with tc.For_i(0, loop_k, 1):
                    emit_body()
            else:
                emit_body()

    return nc


_NC_CACHE: bass.Bass | None = None


def _get_nc() -> bass.Bass:
    global _NC_CACHE
    if _NC_CACHE is None:
        _NC_CACHE = build_kernel()
    return _NC_CACHE


def _make_in_maps(policy_output, target_probs, target_boxes, target_mask):
    po = np.ascontiguousarray(np.asarray(policy_output, dtype=np.float32))
    probs = np.ascontiguousarray(np.asarray(target_probs, dtype=np.float32))
    boxes = np.ascontiguousarray(np.asarray(target_boxes, dtype=np.int32))
    mask = np.ascontiguousarray(np.asarray(target_mask).astype(np.uint8))
    in_maps = []
    for c in range(N_CORES):
        sl = slice(c * BS, (c + 1) * BS)
        in_maps.append(
            {
                "po": po[sl],
                "boxes": boxes[sl],
                "probs": probs[sl],
                "mask": mask[sl],
            }
        )
    return in_maps


def kernel_run(policy_output, target_probs, target_boxes, target_mask, **run_kwargs):
    """Run on 8 cores; returns (loss_scalar_f32, BassKernelResults)."""
    nc = _get_nc()
    in_maps = _make_in_maps(policy_output, target_probs, target_boxes, target_mask)
    res = run_bass_kernel_spmd(nc, in_maps, core_ids=list(range(N_CORES)), **run_kwargs)
    partials = [float(res.results[c]["out"][0, 0]) for c in range(N_CORES)]
    total = float(np.sum(np.array(partials, dtype=np.float64)))
    loss = np.float32(total / NORM)
    return loss, res


def kernel(policy_output, target_probs, target_boxes, target_mask):
    loss, _ = kernel_run(policy_output, target_probs, target_boxes, target_mask)
    return np.asarray(loss, dtype=np.float32)
